# revision 12
# baseline (speedup 1.0000x reference)
"""Trainium2 Bass kernel for nn_AttentionBlock (column-softmax causal attention).

Reference computation (B=4, S=4096, D=128, K=64, V=128):
    Q = x @ Wq.T + bq            [B,S,64]
    Km = x @ Wk.T + bk           [B,S,64]
    Vm = x @ Wv.T + bv           [B,S,128]
    s  = Q @ Km.T / 8            [B,S,S], causal mask j>q -> -1e9
    p  = softmax(s, axis=1)      (softmax over the QUERY axis -- column softmax)
    att = p @ Vm                 [B,S,128]
    out = concat(x, att, dim=2)  [B,S,256]

Key observation: with ST = s.T (layout [j, q]) the softmax denominator
l[j] = sum_q exp(ST[j, q]) is a free-dim reduction, so
att[q] = sum_j exp(ST[j,q]) * (Vm[j]/l[j]) -- a flash-style two-phase kernel
with NO max subtraction needed (scores are O(+-20), exp is safe in fp32).

Sharding (8 cores): core c -> batch b = c//2, j-tile parity p = c%2.
Each core computes l[j] and the PV partial sum for its 16 j-tiles
(j-tile J = 2*i + p), over all q. Host adds the two partials per batch.
All parity differences are data-driven (xkv row gather + additive mask
input), so one SPMD program serves all 8 cores.
"""

import numpy as np

B, S, D = 4, 4096, 128
KD, VD = 64, 128
P = 128
NCORES = 8
JT = 16           # local j-tiles per core
NT = S // P       # 32 global q/j tiles
CHUNK = 1024      # ACT exp chunk width (PSUM cols)

# Matmul dtype for the projections + QK scores: float32r runs the PE at
# 1 cyc/row (vs 4 for plain fp32) when the moving dim is >= 256.
QK_F32R = True

ROW_W = [S - 2 * i * P for i in range(JT)]          # E row widths
EOFF = [0] * JT
for _i in range(1, JT):
    EOFF[_i] = EOFF[_i - 1] + ROW_W[_i - 1]
ECOLS = EOFF[-1] + ROW_W[-1]                        # 34816

_CACHE = {}


def _build_program():
    from contextlib import ExitStack

    from concourse import bacc, bass, mybir  # noqa: F401
    from concourse import tile as tile_mod

    dt = mybir.dt
    f32, bf16 = dt.float32, dt.bfloat16
    f32r = dt.float32r
    Alu = mybir.AluOpType
    ActF = mybir.ActivationFunctionType

    nc = bacc.Bacc(
        "TRN2", target_bir_lowering=False, debug=False, num_devices=NCORES
    )

    x_d = nc.dram_tensor("x", [S, D], f32, kind="ExternalInput").ap()
    xkv_d = nc.dram_tensor("xkv", [JT * P, D], f32, kind="ExternalInput").ap()
    wq_d = nc.dram_tensor("wq", [KD, D], f32, kind="ExternalInput").ap()
    bq_d = nc.dram_tensor("bq", [KD, 1], f32, kind="ExternalInput").ap()
    wk_d = nc.dram_tensor("wk", [KD, D], f32, kind="ExternalInput").ap()
    bk_d = nc.dram_tensor("bk", [KD, 1], f32, kind="ExternalInput").ap()
    wv_d = nc.dram_tensor("wv", [VD, D], f32, kind="ExternalInput").ap()
    bv_d = nc.dram_tensor("bv", [VD, 1], f32, kind="ExternalInput").ap()
    mrow_d = nc.dram_tensor("mrow", [P, 2 * P], f32, kind="ExternalInput").ap()
    att_d = nc.dram_tensor("att", [S, VD], f32, kind="ExternalOutput").ap()

    # Operand tiles of f32r matmuls must be PRODUCED as float32r (the BIR
    # verifier requires the producing instruction to round). All these are
    # written by DVE ops, which round on output.
    mmdt = f32r if QK_F32R else f32

    def mm_cast(ap):
        return ap

    with tile_mod.TileContext(nc) as tc, ExitStack() as ctx:
        persist = ctx.enter_context(tc.tile_pool(name="persist", bufs=1))

        x_sb = persist.tile([P, NT, D], f32)
        xkv_sb = persist.tile([P, JT, D], f32)
        xT = persist.tile([P, S], mmdt)           # [d, q]
        xkvT = persist.tile([P, JT * P], mmdt)    # [d, local j]
        QT = persist.tile([KD, S], mmdt)          # [k, q]
        KTl = persist.tile([KD, JT * P], mmdt)    # [k, local j]
        V_sb = persist.tile([P, JT, VD], f32)     # [local j, v]
        Vp_sb = persist.tile([P, JT, VD], bf16)   # V / l
        E_all = persist.tile([P, ECOLS], bf16)    # exp(scores.T) rows
        l_all = persist.tile([P, JT], f32)
        linv = persist.tile([P, JT], f32)
        wq_sb = persist.tile([KD, D], f32)
        wk_sb = persist.tile([KD, D], f32)
        wv_sb = persist.tile([VD, D], f32)
        WqT = persist.tile([P, KD], mmdt)
        WkT = persist.tile([P, KD], mmdt)
        WvT = persist.tile([P, VD], mmdt)
        VT_sb = persist.tile([P, JT * P], f32)    # [v, local j]
        bq_sb = persist.tile([KD, 1], f32)
        bk_sb = persist.tile([KD, 1], f32)
        bv_sb = persist.tile([VD, 1], f32)
        mrow_sb = persist.tile([P, 2 * P], f32)
        ident = persist.tile([P, P], f32)

        # ---- input DMAs -------------------------------------------------
        nc.sync.dma_start(out=x_sb, in_=x_d.rearrange("(t p) d -> p t d", p=P))
        nc.sync.dma_start(
            out=xkv_sb, in_=xkv_d.rearrange("(t p) d -> p t d", p=P)
        )
        nc.sync.dma_start(out=wq_sb, in_=wq_d)
        nc.sync.dma_start(out=wk_sb, in_=wk_d)
        nc.sync.dma_start(out=wv_sb, in_=wv_d)
        nc.sync.dma_start(out=bq_sb, in_=bq_d)
        nc.sync.dma_start(out=bk_sb, in_=bk_d)
        nc.sync.dma_start(out=bv_sb, in_=bv_d)
        nc.sync.dma_start(out=mrow_sb, in_=mrow_d)

        # identity for PE transposes
        nc.gpsimd.memset(ident, 0.0)
        nc.gpsimd.affine_select(
            out=ident,
            in_=ident,
            compare_op=Alu.not_equal,
            fill=1.0,
            base=0,
            pattern=[[-1, P]],
            channel_multiplier=1,
        )

        # ---- phase A: transposes + projections --------------------------
        with ExitStack() as pha:
            tpp = pha.enter_context(
                tc.tile_pool(name="tp_psum", bufs=2, space="PSUM")
            )
            prj = pha.enter_context(
                tc.tile_pool(name="prj_psum", bufs=2, space="PSUM")
            )

            # x^T (for QT), xkv^T (for KT/VT): PE transposes, 4 tiles per
            # PSUM bank then one DVE copy per bank.
            for grp in range(NT // 4):
                ps = tpp.tile([P, 4, P], f32, tag="tp4")
                for k in range(4):
                    t = grp * 4 + k
                    nc.tensor.transpose(ps[:, k, :], x_sb[:, t, :], ident)
                nc.vector.tensor_copy(
                    xT[:, grp * 4 * P : (grp + 1) * 4 * P],
                    ps.rearrange("p a b -> p (a b)"),
                )
            for grp in range(JT // 4):
                ps = tpp.tile([P, 4, P], f32, tag="tp4")
                for k in range(4):
                    t = grp * 4 + k
                    nc.tensor.transpose(ps[:, k, :], xkv_sb[:, t, :], ident)
                nc.vector.tensor_copy(
                    xkvT[:, grp * 4 * P : (grp + 1) * 4 * P],
                    ps.rearrange("p a b -> p (a b)"),
                )

            # weight transposes (share the tp4 slot shape)
            psw = tpp.tile([P, 4, P], f32, tag="tp4")
            nc.tensor.transpose(psw[:, 0, :KD], wq_sb, ident[:KD, :KD])
            nc.tensor.transpose(psw[:, 1, :KD], wk_sb, ident[:KD, :KD])
            nc.tensor.transpose(psw[:, 2, :], wv_sb, ident)
            nc.vector.tensor_copy(WqT, psw[:, 0, :KD])
            nc.vector.tensor_copy(WkT, psw[:, 1, :KD])
            nc.vector.tensor_copy(WvT, psw[:, 2, :])

            # QT = Wq_s @ x^T  (+bq per-partition, fused into the copy)
            for c in range(S // 512):
                ps = prj.tile([P, 512], f32, tag="prj")
                nc.tensor.matmul(
                    ps[:KD, :],
                    lhsT=mm_cast(WqT),
                    rhs=mm_cast(xT[:, c * 512 : (c + 1) * 512]),
                    start=True,
                    stop=True,
                )
                nc.vector.tensor_scalar(
                    out=QT[:, c * 512 : (c + 1) * 512],
                    in0=ps[:KD, :],
                    scalar1=bq_sb,
                    scalar2=None,
                    op0=Alu.add,
                )
            # KT local = Wk @ xkv^T (+bk)
            for c in range(JT * P // 512):
                ps = prj.tile([P, 512], f32, tag="prj")
                nc.tensor.matmul(
                    ps[:KD, :],
                    lhsT=mm_cast(WkT),
                    rhs=mm_cast(xkvT[:, c * 512 : (c + 1) * 512]),
                    start=True,
                    stop=True,
                )
                nc.vector.tensor_scalar(
                    out=KTl[:, c * 512 : (c + 1) * 512],
                    in0=ps[:KD, :],
                    scalar1=bk_sb,
                    scalar2=None,
                    op0=Alu.add,
                )
            # VT = Wv @ xkv^T (+bv), then per-tile transpose -> V [j, v]
            for c in range(JT * P // 512):
                ps = prj.tile([P, 512], f32, tag="prj")
                nc.tensor.matmul(
                    ps,
                    lhsT=mm_cast(WvT),
                    rhs=mm_cast(xkvT[:, c * 512 : (c + 1) * 512]),
                    start=True,
                    stop=True,
                )
                nc.vector.tensor_scalar(
                    out=VT_sb[:, c * 512 : (c + 1) * 512],
                    in0=ps,
                    scalar1=bv_sb,
                    scalar2=None,
                    op0=Alu.add,
                )
            for grp in range(JT // 4):
                ps = tpp.tile([P, 4, P], f32, tag="tp4")
                for k in range(4):
                    i = grp * 4 + k
                    nc.tensor.transpose(
                        ps[:, k, :], VT_sb[:, i * P : (i + 1) * P], ident
                    )
                nc.vector.tensor_copy(
                    V_sb[:, grp * 4 : (grp + 1) * 4, :],
                    ps,
                )

        # ---- phase B (rows: QK + exp + stats) & phase C (PV), interleaved
        with ExitStack() as phb:
            rowp = phb.enter_context(
                tc.tile_pool(name="row_psum", bufs=2, space="PSUM")
            )
            attp = phb.enter_context(
                tc.tile_pool(name="att_psum", bufs=2, space="PSUM")
            )
            sbo = phb.enter_context(tc.tile_pool(name="att_sb", bufs=2))
            lpp = phb.enter_context(tc.tile_pool(name="lparts", bufs=8))

            def emit_pv_group(g):
                aps = attp.tile([P, 8, VD], f32, tag="att")
                for k in range(8):
                    tq = 8 * g + k
                    rows = [i for i in range(JT) if 2 * i <= tq]
                    for ridx, i in enumerate(rows):
                        ecol = EOFF[i] + (tq - 2 * i) * P
                        nc.tensor.matmul(
                            aps[:, k, :],
                            lhsT=E_all[:, ecol : ecol + P],
                            rhs=Vp_sb[:, i, :],
                            start=(ridx == 0),
                            stop=(ridx == len(rows) - 1),
                        )
                sb = sbo.tile([P, 8, VD], f32, tag="osb")
                nc.vector.tensor_copy(sb, aps)
                nc.sync.dma_start(
                    out=att_d[g * 8 * P : (g + 1) * 8 * P, :].rearrange(
                        "(t p) v -> p t v", p=P
                    ),
                    in_=sb,
                )

            for i in range(JT):
                q0 = 2 * i * P
                w = ROW_W[i]
                nchunks = (w + CHUNK - 1) // CHUNK
                for c in range(nchunks):
                    cw = min(CHUNK, w - c * CHUNK)
                    ps = rowp.tile([P, CHUNK], f32, tag="st")
                    off = q0 + c * CHUNK
                    for s0 in range(0, cw, 512):
                        sw = min(512, cw - s0)
                        nc.tensor.matmul(
                            ps[:, s0 : s0 + sw],
                            lhsT=mm_cast(KTl[:, i * P : (i + 1) * P]),
                            rhs=mm_cast(QT[:, off + s0 : off + s0 + sw]),
                            start=True,
                            stop=True,
                        )
                    if c == 0:
                        # additive causal mask on the first two q-tiles
                        nc.vector.tensor_add(ps[:, : 2 * P], ps[:, : 2 * P], mrow_sb)
                    lp = lpp.tile([P, 1], f32, tag="lp")
                    ecol = EOFF[i] + c * CHUNK
                    nc.scalar.activation(
                        out=E_all[:, ecol : ecol + cw],
                        in_=ps[:, :cw],
                        func=ActF.Exp,
                        accum_out=lp,
                    )
                    if c == 0:
                        nc.vector.tensor_copy(l_all[:, i : i + 1], lp)
                    else:
                        nc.vector.tensor_add(
                            l_all[:, i : i + 1], l_all[:, i : i + 1], lp
                        )
                nc.vector.reciprocal(linv[:, i : i + 1], l_all[:, i : i + 1])
                nc.vector.tensor_scalar(
                    out=Vp_sb[:, i, :],
                    in0=V_sb[:, i, :],
                    scalar1=linv[:, i : i + 1],
                    scalar2=None,
                    op0=Alu.mult,
                )
                if i % 4 == 3:
                    emit_pv_group(i // 4)

    nc.compile()
    return nc


def _host_inputs(x, Wq, bq, Wk, bk, Wv, bv):
    """Per-core input maps."""
    x_full = np.ascontiguousarray(x, dtype=np.float32)
    Wq_s = (np.asarray(Wq, np.float32) / 8.0).astype(np.float32)
    bq_s = (np.asarray(bq, np.float32) / 8.0).reshape(KD, 1).astype(np.float32)
    Wk_ = np.ascontiguousarray(Wk, dtype=np.float32)
    bk_ = np.asarray(bk, np.float32).reshape(KD, 1)
    Wv_ = np.ascontiguousarray(Wv, dtype=np.float32)
    bv_ = np.asarray(bv, np.float32).reshape(VD, 1)

    tri = np.where(
        np.arange(P)[None, :] >= np.arange(P)[:, None], 0.0, -1e9
    ).astype(np.float32)
    mrows = []
    for p in (0, 1):
        m = np.zeros((P, 2 * P), np.float32)
        if p == 0:
            m[:, :P] = tri
        else:
            m[:, :P] = -1e9
            m[:, P:] = tri
        mrows.append(m)

    in_maps = []
    for c in range(NCORES):
        b, p = c // 2, c % 2
        xb = x_full[b]
        xkv = np.ascontiguousarray(
            xb.reshape(NT, P, D)[p::2].reshape(JT * P, D)
        )
        in_maps.append(
            {
                "x": xb,
                "xkv": xkv,
                "wq": Wq_s,
                "bq": bq_s,
                "wk": Wk_,
                "bk": bk_,
                "wv": Wv_,
                "bv": bv_,
                "mrow": mrows[p],
            }
        )
    return in_maps


def _get_program():
    if "nc" not in _CACHE:
        _CACHE["nc"] = _build_program()
    return _CACHE["nc"]


def run_on_device(in_maps, trace=False, trace_kwargs=None):
    from concourse import bass_utils

    nc = _get_program()
    return bass_utils.run_bass_kernel_spmd(
        nc,
        in_maps,
        core_ids=list(range(NCORES)),
        trace=trace,
        trace_kwargs=trace_kwargs or {},
    )


def kernel(x, Wq, bq, Wk, bk, Wv, bv):
    x = np.asarray(x, np.float32)
    in_maps = _host_inputs(x, Wq, bq, Wk, bk, Wv, bv)
    res = run_on_device(in_maps)
    att = np.empty((B, S, VD), np.float32)
    for b in range(B):
        att[b] = res.results[2 * b]["att"] + res.results[2 * b + 1]["att"]
    return np.concatenate([x, att], axis=2)
